# revision 22
# baseline (speedup 1.0000x reference)
"""Trainium2 Bass kernel for nn_MultiHeadedAttention_6416681140387.

Two-branch windowed video attention:
  x [8,256,96,96] -> 1x1 conv Q/K/V -> per-branch full attention over
  window-token features (branch0: 4x4 patches, d=2048, 2304 key tokens;
  branch1: 8x8 patches, d=8192, 576 key tokens) -> concat channels
  -> 3x3 conv + LeakyReLU(0.2).

Sharding: 8 cores = (video b in {0,1}) x (frame t in {0..3}). Each core
receives ONLY its own frame, computes K/V (and Q) convs for that frame,
then 4-core AllGather collectives ([0..3], [4..7]) share the window-major
K tiles and [token, d] V tiles across the video group. Scores/softmax/PV
and the 3x3 output conv run per-core on the gathered data.

Design (gather-once, window-major convs):
  x is gathered once per branch into window-major xw (fp16); Q/K/V
  1x1-convs all consume xw, so K comes out window-major and the scores
  matmuls get contiguous rhs. V is produced in [token, d] layout directly
  (stationary-xw trick). Whole 16-bit path is fp16.
"""

import sys

if "/opt/trn_rl_repo" not in sys.path:
    sys.path.insert(0, "/opt/trn_rl_repo")

import math
from contextlib import ExitStack

import numpy as np

import concourse.bass as bass
import concourse.tile as tile
from concourse import bacc, mybir
from concourse.masks import make_identity

F32 = mybir.dt.float32
FP16 = mybir.dt.float16

T = 4
C = 256
H = W = 96
PIX = H * W
NCORES = 8

PSZ = [4, 8]
OHB = [24, 12]                  # token grid side per branch
NTF = [576, 144]                # tokens per frame
NKP = [2304, 576]               # key tokens per video
NCH = [16, 64]                  # d-chunks (psz^2)
NTILE = [18, 5]                 # ceil(NKP/128)
SC = [1.0 / math.sqrt(2048.0), 1.0 / math.sqrt(8192.0)]
NQB = [[(0, 128), (128, 128), (256, 128), (384, 128), (512, 64)],
       [(0, 128), (128, 16)]]
RG = [[0, 1, 2, 3], [4, 5, 6, 7]]

Exp = mybir.ActivationFunctionType.Exp
Identity = mybir.ActivationFunctionType.Identity


def build(nc):
    xf = nc.dram_tensor("xf", [C, PIX], F32, kind="ExternalInput")
    wqt = nc.dram_tensor("wqt", [C, C], F32, kind="ExternalInput")
    wkt = nc.dram_tensor("wkt", [C, C], F32, kind="ExternalInput")
    wvt = nc.dram_tensor("wvt", [C, C], F32, kind="ExternalInput")
    wot = nc.dram_tensor("wot", [9, C, C], F32, kind="ExternalInput")
    bq = nc.dram_tensor("bq", [C], F32, kind="ExternalInput")
    bk = nc.dram_tensor("bk", [C], F32, kind="ExternalInput")
    bv = nc.dram_tensor("bv", [C], F32, kind="ExternalInput")
    bo = nc.dram_tensor("bo", [C], F32, kind="ExternalInput")
    out = nc.dram_tensor("out", [C, PIX], F32, kind="ExternalOutput")

    alt = [0]

    def bias_copy_alt(dst, src, bias_ap):
        alt[0] ^= 1
        if alt[0]:
            nc.scalar.activation(out=dst, in_=src, func=Identity,
                                 bias=bias_ap, scale=1.0)
        else:
            nc.vector.tensor_scalar_add(dst, src, bias_ap)

    rr = [0]

    def copy_rr(dst, src):
        rr[0] ^= 1
        if rr[0]:
            nc.vector.tensor_copy(dst, src)
        else:
            nc.scalar.copy(dst, src)

    sv = [0]

    def copy_sv(dst, src):
        sv[0] ^= 1
        if sv[0]:
            nc.scalar.copy(dst, src)
        else:
            nc.vector.tensor_copy(dst, src)

    with tile.TileContext(nc, pool_alloc_mode="queue") as tc, ExitStack() as top:
        persist = top.enter_context(tc.tile_pool(name="persist", bufs=1))
        dramp = top.enter_context(tc.tile_pool(name="dram", bufs=1, space="DRAM"))

        wq_sb, wk_sb, wv_sb = [None, None], [None, None], [None, None]
        with tc.tile_pool(name="wload", bufs=2) as p_wl:
            for name, dt_, lst in (("wq", wqt, wq_sb), ("wk", wkt, wk_sb),
                                   ("wv", wvt, wv_sb)):
                for cb in range(2):
                    tf = p_wl.tile([128, C], F32, name="wl", tag="wl")
                    nc.sync.dma_start(out=tf,
                                      in_=dt_.ap()[cb * 128:(cb + 1) * 128, :])
                    t = persist.tile([128, C], FP16, name=f"{name}{cb}",
                                     tag=f"{name}{cb}")
                    nc.vector.tensor_copy(t, tf)
                    lst[cb] = t

        def bias_tile(name, dt_):
            t = persist.tile([128, 2], F32, name=name, tag=name)
            nc.sync.dma_start(
                out=t, in_=bass.AP(tensor=dt_.ap().tensor, offset=0,
                                   ap=[[1, 128], [128, 2]]))
            return t

        bq_sb = bias_tile("bq", bq)
        bk_sb = bias_tile("bk", bk)
        bo_sb = bias_tile("bo", bo)
        bv_sb = bias_tile("bv", bv)
        ident = persist.tile([128, 128], FP16, name="ident", tag="ident")
        make_identity(nc, ident)
        zrow = persist.tile([128, 98], FP16, name="zrow", tag="zrow")
        nc.vector.memset(zrow, 0.0)

        # collective buffers (DRAM): own-frame contribution -> video gather
        kwcc_in = [dramp.tile([128, NCH[b] * NTF[b]], FP16,
                              name=f"kwi{b}", tag=f"kwi{b}") for b in range(2)]
        kwcc_out = [dramp.tile([T * 128, NCH[b] * NTF[b]], FP16,
                               name=f"kwo{b}", tag=f"kwo{b}") for b in range(2)]
        vcc_in = [dramp.tile([NTF[b], NCH[b] * 128], FP16,
                             name=f"vi{b}", tag=f"vi{b}") for b in range(2)]
        vcc_out = [dramp.tile([NKP[b], NCH[b] * 128], FP16,
                              name=f"vo{b}", tag=f"vo{b}") for b in range(2)]
        att0_dram = dramp.tile([128, 98 * 98], FP16, name="att0d", tag="att0d")
        qw_dram = [dramp.tile([128, NCH[b] * NTF[b]], FP16, name=f"qwd{b}",
                              tag=f"qwd{b}") for b in range(2)]

        # ---------------- conv phase: own frame only ----------------
        with tc.tile_pool(name="xc", bufs=2) as p_xc, \
             tc.tile_pool(name="xw", bufs=2) as p_xw, \
             tc.tile_pool(name="ks", bufs=4) as p_ks, \
             tc.tile_pool(name="vs", bufs=4) as p_vs, \
             tc.tile_pool(name="cps", bufs=3, space="PSUM") as p_cps, \
             tc.tile_pool(name="vps", bufs=3, space="PSUM") as p_vps:
            for br in range(2):
                psz, ohb, ntf, nch = PSZ[br], OHB[br], NTF[br], NCH[br]
                ohc = 3 if br == 0 else 2
                nql = ohb // ohc
                csz = ohc * psz * W

                xw = [p_xw.tile([128, nch * ntf], FP16, name=f"xw{cb}",
                                tag=f"xw{cb}") for cb in range(2)]
                xwv = [xw[cb].rearrange(
                    "p (wy wx oh ow) -> p wy wx oh ow",
                    wy=psz, wx=psz, oh=ohb, ow=ohb) for cb in range(2)]
                for chq in range(nql):
                    for cb in range(2):
                        xc = p_xc.tile([128, csz], F32, name=f"xc{cb}",
                                       tag=f"xc{cb}")
                        nc.sync.dma_start(
                            out=xc,
                            in_=xf.ap()[cb * 128:(cb + 1) * 128,
                                        chq * csz:(chq + 1) * csz])
                        xcv = xc.rearrange(
                            "p (oh hh ow ww) -> p oh hh ow ww",
                            oh=ohc, hh=psz, ow=ohb, ww=psz)
                        for wy in range(psz):
                            src = xcv[:, :, wy, :, :].rearrange(
                                "p a b c -> p c a b")
                            dst = xwv[cb][:, wy, :,
                                          chq * ohc:(chq + 1) * ohc, :]
                            copy_rr(dst, src)

                # Q conv -> qw_dram; K conv -> kwcc_in (DRAM)
                for k in range(nch * ntf // 512):
                    psq = p_cps.tile([128, 512], F32, name="cps", tag="cps")
                    for cb in range(2):
                        nc.tensor.matmul(
                            psq, wq_sb[cb][:, br * 128:(br + 1) * 128],
                            xw[cb][:, k * 512:(k + 1) * 512],
                            start=(cb == 0), stop=(cb == 1))
                    qst = p_ks.tile([128, 512], FP16, name="qs", tag="ks")
                    bias_copy_alt(qst, psq, bq_sb[:, br:br + 1])
                    nc.gpsimd.dma_start(
                        out=qw_dram[br][:, k * 512:(k + 1) * 512], in_=qst)
                    psk = p_cps.tile([128, 512], F32, name="cps", tag="cps")
                    for cb in range(2):
                        nc.tensor.matmul(
                            psk, wk_sb[cb][:, br * 128:(br + 1) * 128],
                            xw[cb][:, k * 512:(k + 1) * 512],
                            start=(cb == 0), stop=(cb == 1))
                    kst = p_ks.tile([128, 512], FP16, name="ks", tag="ks")
                    bias_copy_alt(kst, psk, bk_sb[:, br:br + 1])
                    nc.gpsimd.dma_start(
                        out=kwcc_in[br][:, k * 512:(k + 1) * 512], in_=kst)

                # V conv -> vcc_in (DRAM), [token, d] layout
                for gi, (g0, gm) in enumerate(NQB[br]):
                    for cq in range(nch // 4):
                        ps = p_vps.tile([128, 512], F32, name="vps",
                                        tag="vps")
                        for cis4 in range(4):
                            ci = cq * 4 + cis4
                            for cb in range(2):
                                nc.tensor.matmul(
                                    ps[:gm, cis4 * 128:(cis4 + 1) * 128],
                                    xw[cb][:, ci * ntf + g0:
                                           ci * ntf + g0 + gm],
                                    wv_sb[cb][:, br * 128:(br + 1) * 128],
                                    start=(cb == 0), stop=(cb == 1))
                        vst = p_vs.tile([128, 512], FP16, name="vs", tag="vs")
                        copy_sv(vst[:gm, :], ps[:gm, :])
                        nc.gpsimd.dma_start(
                            out=vcc_in[br][g0:g0 + gm,
                                           cq * 512:(cq + 1) * 512],
                            in_=vst[:gm, :])

        # ---------------- AllGather across the video group ----------------
        for br in range(2):
            nc.gpsimd.collective_compute(
                "AllGather", mybir.AluOpType.bypass, RG,
                ins=[kwcc_in[br]], outs=[kwcc_out[br]])
            nc.gpsimd.collective_compute(
                "AllGather", mybir.AluOpType.bypass, RG,
                ins=[vcc_in[br]], outs=[vcc_out[br]])

        # att1 outlives pass 1 (used by phase D)
        esAtt1 = ExitStack()
        p_att1 = esAtt1.enter_context(tc.tile_pool(name="att1", bufs=1))
        att_sb = {}

        for br in range(2):
            psz, ohb, ntf, nch = PSZ[br], OHB[br], NTF[br], NCH[br]
            nkp, ntile = NKP[br], NTILE[br]
            nmk = 2 if br == 0 else 1
            mkw = ntf // nmk

            es_pt = ExitStack()
            p_pt = es_pt.enter_context(tc.tile_pool(name=f"pt{br}", bufs=1))
            pt_t = [p_pt.tile([128, ntf], FP16, name=f"pt{br}_{i}",
                              tag=f"pt{br}_{i}") for i in range(ntile)]
            es_P = ExitStack()
            p_P = es_P.enter_context(tc.tile_pool(name=f"P{br}", bufs=1))
            p_t = [p_P.tile([128, nkp], FP16, name=f"p{br}_{i}",
                            tag=f"p{br}_{i}") for i in range(len(NQB[br]))]
            p_run = es_P.enter_context(tc.tile_pool(name=f"run{br}", bufs=1))
            run_mx = [p_run.tile([128, 1], F32, name=f"mx{i}", tag=f"mx{i}")
                      for i in range(len(NQB[br]))]
            run_ls = [p_run.tile([128, 1], F32, name=f"ls{i}", tag=f"ls{i}")
                      for i in range(len(NQB[br]))]

            # ---- scores + online softmax, streaming kw per key-frame ----
            with tc.tile_pool(name=f"qwp{br}", bufs=1) as p_qwp, \
                 tc.tile_pool(name=f"kwl{br}", bufs=2) as p_kwl, \
                 tc.tile_pool(name=f"st{br}", bufs=4) as p_stat, \
                 tc.tile_pool(name=f"sps{br}", bufs=2, space="PSUM") as p_sps:
                qwt = p_qwp.tile([128, nch * ntf], FP16, name="qwt",
                                 tag="qwt")
                nc.sync.dma_start(out=qwt, in_=qw_dram[br])
                for kf in range(T):
                    kw = p_kwl.tile([128, nch * ntf], FP16, name="kwl",
                                    tag="kwl")
                    nc.sync.dma_start(
                        out=kw, in_=kwcc_out[br][kf * 128:(kf + 1) * 128, :])
                    for nqi, (q0, nqsz) in enumerate(NQB[br]):
                        pss = [p_sps.tile([128, mkw], F32, name=f"s{mkh}",
                                          tag=f"s{mkh}")
                               for mkh in range(nmk)]
                        for ci in range(nch):
                            for mkh in range(nmk):
                                nc.tensor.matmul(
                                    pss[mkh][:nqsz],
                                    qwt[:, ci * ntf + q0:
                                        ci * ntf + q0 + nqsz],
                                    kw[:, ci * ntf + mkh * mkw:
                                       ci * ntf + (mkh + 1) * mkw],
                                    start=(ci == 0), stop=(ci == nch - 1))
                        for mkh in range(nmk):
                            ps = pss[mkh]
                            o = kf * ntf + mkh * mkw
                            pt = p_t[nqi]
                            mx, ls = run_mx[nqi], run_ls[nqi]
                            bm = p_stat.tile([128, 1], F32, name="bm",
                                             tag="bm")
                            nc.vector.reduce_max(out=bm[:nqsz],
                                                 in_=ps[:nqsz, :],
                                                 axis=mybir.AxisListType.X)
                            if kf == 0 and mkh == 0:
                                nc.vector.tensor_copy(mx[:nqsz], bm[:nqsz])
                                nmx = p_stat.tile([128, 1], F32, name="nmx",
                                                  tag="nmx")
                                nc.vector.tensor_scalar_mul(
                                    nmx[:nqsz], mx[:nqsz], -SC[br])
                                nc.scalar.activation(
                                    out=pt[:nqsz, o:o + mkw],
                                    in_=ps[:nqsz, :], func=Exp,
                                    bias=nmx[:nqsz], scale=SC[br],
                                    accum_out=ls[:nqsz])
                            else:
                                nmax = p_stat.tile([128, 1], F32,
                                                   name="nmax", tag="nmax")
                                nc.vector.tensor_max(nmax[:nqsz], mx[:nqsz],
                                                     bm[:nqsz])
                                nmx = p_stat.tile([128, 1], F32, name="nmx",
                                                  tag="nmx")
                                nc.vector.tensor_scalar_mul(
                                    nmx[:nqsz], nmax[:nqsz], -SC[br])
                                delta = p_stat.tile([128, 1], F32,
                                                    name="delta", tag="delta")
                                nc.scalar.activation(
                                    out=delta[:nqsz], in_=mx[:nqsz],
                                    func=Exp, bias=nmx[:nqsz], scale=SC[br])
                                nc.vector.tensor_scalar_mul(
                                    pt[:nqsz, 0:o], pt[:nqsz, 0:o],
                                    delta[:nqsz])
                                pl = p_stat.tile([128, 1], F32, name="pl",
                                                 tag="pl")
                                nc.scalar.activation(
                                    out=pt[:nqsz, o:o + mkw],
                                    in_=ps[:nqsz, :], func=Exp,
                                    bias=nmx[:nqsz], scale=SC[br],
                                    accum_out=pl[:nqsz])
                                nc.vector.scalar_tensor_tensor(
                                    out=ls[:nqsz], in0=ls[:nqsz],
                                    scalar=delta[:nqsz], in1=pl[:nqsz],
                                    op0=mybir.AluOpType.mult,
                                    op1=mybir.AluOpType.add)
                                nc.vector.tensor_copy(mx[:nqsz], nmax[:nqsz])

                # finalize softmax: P /= ls
                for nqi, (q0, nqsz) in enumerate(NQB[br]):
                    rs = p_stat.tile([128, 1], F32, name="rs", tag="rs")
                    nc.vector.reciprocal(rs[:nqsz], run_ls[nqi][:nqsz])
                    nc.vector.tensor_scalar_mul(
                        p_t[nqi][:nqsz, :], p_t[nqi][:nqsz, :], rs[:nqsz])

            # ---- P^T transposes ----
            with tc.tile_pool(name=f"tp{br}", bufs=2, space="PSUM") as p_tp:
                for ti in range(ntile):
                    t0 = ti * 128
                    m = min(128, nkp - t0)
                    for nqi, (q0, nqsz) in enumerate(NQB[br]):
                        tp = p_tp.tile([128, 128], FP16, name="tp", tag="tp")
                        nc.tensor.transpose(
                            tp[:m, :nqsz], p_t[nqi][:nqsz, t0:t0 + m],
                            ident[:nqsz, :nqsz])
                        copy_sv(pt_t[ti][:m, q0:q0 + nqsz], tp[:m, :nqsz])
            es_P.close()

            # ---- PV: y^T accumulated over key tiles -> att ----
            es_att0 = ExitStack()
            if br == 0:
                p_att = es_att0.enter_context(
                    tc.tile_pool(name="att0", bufs=1))
            else:
                p_att = p_att1
            att = p_att.tile([128, 98 * 98], FP16, name=f"att{br}",
                             tag=f"att{br}")
            att_sb[br] = att
            attv = att.rearrange("p (h w) -> p h w", h=98)
            nc.scalar.copy(att[:, 0:98], zrow)
            nc.scalar.copy(att[:, 97 * 98:98 * 98], zrow)
            zcol = zrow[:, 0:96].rearrange("p (a c) -> p a c", a=96)
            nc.vector.tensor_copy(attv[:, 1:97, 0:1], zcol)
            nc.vector.tensor_copy(attv[:, 1:97, 97:98], zcol)
            wvw = attv[:, 1:97, 1:97].rearrange(
                "p (oh hh) (ow ww) -> p oh hh ow ww", hh=psz, ww=psz)

            es_vt = ExitStack()
            # split across 3 pools: the queue-ring allocator needs gaps
            p_vts = [es_vt.enter_context(
                tc.tile_pool(name=f"vt{br}_{h}", bufs=1)) for h in range(3)]
            vt = []
            for ti in range(ntile):
                t0 = ti * 128
                m = min(128, nkp - t0)
                t = p_vts[ti % 3].tile([128, nch * 128], FP16,
                                       name=f"vt{ti}", tag=f"vt{ti}")
                nc.gpsimd.dma_start(out=t[:m, :], in_=vcc_out[br][t0:t0 + m, :])
                vt.append(t)

            nqh_n = 2 if br == 0 else 1
            nqw = ntf // nqh_n
            ohq = ohb // nqh_n
            with tc.tile_pool(name=f"pv{br}", bufs=2, space="PSUM") as p_pv:
                for ci in range(nch):
                    wy, wx = divmod(ci, psz)
                    pss = [p_pv.tile([128, nqw], F32, name=f"pv{nqh}",
                                     tag=f"pv{nqh}")
                           for nqh in range(nqh_n)]
                    for ti in range(ntile):
                        m = min(128, nkp - ti * 128)
                        for nqh in range(nqh_n):
                            nc.tensor.matmul(
                                pss[nqh],
                                vt[ti][:m, ci * 128:(ci + 1) * 128],
                                pt_t[ti][:m, nqh * nqw:(nqh + 1) * nqw],
                                start=(ti == 0), stop=(ti == ntile - 1))
                    for nqh in range(nqh_n):
                        dst = wvw[:, nqh * ohq:(nqh + 1) * ohq, wy, :, wx]
                        src = pss[nqh].rearrange("p (a c) -> p a c", a=ohq)
                        bias_copy_alt(dst, src, bv_sb[:, br:br + 1])
            es_vt.close()
            if br == 0:
                nc.gpsimd.dma_start(out=att0_dram, in_=att)
                es_att0.close()
            es_pt.close()

        # ---------------- phase D: 3x3 conv + LeakyReLU ----------------
        with tc.tile_pool(name="attr", bufs=1) as p_attr, \
             tc.tile_pool(name="wot", bufs=1) as p_wot, \
             tc.tile_pool(name="wotl", bufs=2) as p_wotl, \
             tc.tile_pool(name="dout", bufs=3) as p_do, \
             tc.tile_pool(name="dps", bufs=4, space="PSUM") as p_dps:
            att0 = p_attr.tile([128, 98 * 98], FP16, name="attr0",
                               tag="attr0")
            nc.sync.dma_start(out=att0, in_=att0_dram)
            att_in = [att0, att_sb[1]]
            wot_sb = []
            for cb in range(2):
                tf = p_wotl.tile([128, 9 * C], F32, name="wotl", tag="wotl")
                nc.sync.dma_start(
                    out=tf.rearrange("i (t o) -> i t o", t=9),
                    in_=wot.ap()[:, cb * 128:(cb + 1) * 128, :].rearrange(
                        "t i o -> i t o"))
                t = p_wot.tile([128, 9, C], FP16, name=f"wot{cb}",
                               tag=f"wot{cb}")
                nc.vector.tensor_copy(t, tf.rearrange("i (t o) -> i t o",
                                                      t=9))
                wot_sb.append(t)
            attv2 = [att_in[cb].rearrange("p (h w) -> p h w", h=98)
                     for cb in range(2)]
            for coutb in range(2):
                for rg in range(24):
                    ps = p_dps.tile([128, 384], F32, name="dps", tag="dps")
                    k = 0
                    for cb in range(2):
                        for tap in range(9):
                            dy, dx = divmod(tap, 3)
                            rhs = attv2[cb][:, rg * 4 + dy:rg * 4 + dy + 4,
                                            dx:dx + 96]
                            lhsT = wot_sb[cb][:, tap,
                                              coutb * 128:(coutb + 1) * 128]
                            nc.tensor.matmul(ps, lhsT, rhs,
                                             start=(k == 0), stop=(k == 17))
                            k += 1
                    t1 = p_do.tile([128, 384], F32, name="t1", tag="t1")
                    nc.scalar.activation(out=t1, in_=ps, func=Identity,
                                         bias=bo_sb[:, coutb:coutb + 1],
                                         scale=1.0)
                    t2 = p_do.tile([128, 384], F32, name="t2", tag="t2")
                    nc.vector.scalar_tensor_tensor(
                        out=t2, in0=t1, scalar=0.2, in1=t1,
                        op0=mybir.AluOpType.mult, op1=mybir.AluOpType.max)
                    nc.sync.dma_start(
                        out=out.ap()[coutb * 128:(coutb + 1) * 128,
                                     rg * 384:(rg + 1) * 384],
                        in_=t2)
        esAtt1.close()
    return nc


_CACHED = {}


def _get_nc():
    if "nc" not in _CACHED:
        nc = bacc.Bacc("TRN2", debug=False, target_bir_lowering=False,
                       num_devices=NCORES)
        build(nc)
        nc.compile()
        _CACHED["nc"] = nc
    return _CACHED["nc"]


def make_in_maps(x, wq, bq_, wk, bk_, wv, bv_, wo, bo_):
    shared = {
        "wqt": np.ascontiguousarray(wq.T.astype(np.float32)),
        "wkt": np.ascontiguousarray(wk.T.astype(np.float32)),
        "wvt": np.ascontiguousarray(wv.T.astype(np.float32)),
        "wot": np.ascontiguousarray(
            wo.transpose(2, 3, 1, 0).reshape(9, C, C).astype(np.float32)),
        "bq": np.ascontiguousarray(bq_.astype(np.float32)),
        "bk": np.ascontiguousarray(bk_.astype(np.float32)),
        "bv": np.ascontiguousarray(bv_.astype(np.float32)),
        "bo": np.ascontiguousarray(bo_.astype(np.float32)),
    }
    x3 = np.ascontiguousarray(x.reshape(2 * T, C, PIX).astype(np.float32))
    in_maps = []
    for core in range(NCORES):
        m = dict(shared)
        m["xf"] = np.ascontiguousarray(x3[core])
        in_maps.append(m)
    return in_maps


def kernel(**inputs):
    from concourse.bass_utils import run_bass_kernel_spmd

    x = np.asarray(inputs["x"], dtype=np.float32)
    in_maps = make_in_maps(
        x, np.asarray(inputs["wq"]), np.asarray(inputs["bq"]),
        np.asarray(inputs["wk"]), np.asarray(inputs["bk"]),
        np.asarray(inputs["wv"]), np.asarray(inputs["bv"]),
        np.asarray(inputs["wo"]), np.asarray(inputs["bo"]))
    nc = _get_nc()
    res = run_bass_kernel_spmd(nc, in_maps, core_ids=list(range(NCORES)))
    outs = [res.results[c]["out"].reshape(C, H, W) for c in range(NCORES)]
    return np.stack(outs).astype(np.float32)


# revision 26
# speedup vs baseline: 1.1784x; 1.1784x over previous
"""Trainium2 Bass kernel for nn_MultiHeadedAttention_6416681140387.

Two-branch windowed video attention:
  x [8,256,96,96] -> 1x1 conv Q/K/V -> per-branch full attention over
  window-token features (branch0: 4x4 patches, d=2048, 2304 key tokens;
  branch1: 8x8 patches, d=8192, 576 key tokens) -> concat channels
  -> 3x3 conv + LeakyReLU(0.2).

Sharding: 8 cores = (video b in {0,1}) x (frame t in {0..3}). Each core
computes its full output frame [256,96,96]. The host permutes each core's
4-frame video slice so the core's own frame comes first; attention is
key-order invariant so P columns / V rows just follow processing order.

Design (gather-once, window-major convs):
  Per branch pass, per frame: x is gathered once into window-major xw
  (fp16), and Q (frame 0 only) / K / V 1x1-convs all consume xw. K conv
  output is therefore window-major => scores matmuls get contiguous rhs.
  V tiles ([token, d] layout) are spilled to a DRAM scratch and streamed
  back for the PV phase (all-SBUF working set stays under the 192KB cap).
  Whole 16-bit path is fp16 (better mantissa than bf16, same PE rate).
"""

import sys

if "/opt/trn_rl_repo" not in sys.path:
    sys.path.insert(0, "/opt/trn_rl_repo")

import math
from contextlib import ExitStack

import numpy as np

import concourse.bass as bass
import concourse.tile as tile
from concourse import bacc, mybir
from concourse.masks import make_identity

F32 = mybir.dt.float32
FP16 = mybir.dt.float16

T = 4
C = 256
H = W = 96
PIX = H * W
NCORES = 8

PSZ = [4, 8]
OHB = [24, 12]                  # token grid side per branch
NTF = [576, 144]                # tokens per frame
NKP = [2304, 576]               # key tokens per video (no padding)
NCH = [16, 64]                  # d-chunks (psz^2)
NTILE = [18, 5]                 # ceil(NKP/128)
SC = [1.0 / math.sqrt(2048.0), 1.0 / math.sqrt(8192.0)]
NQB = [[(0, 128), (128, 128), (256, 128), (384, 128), (512, 64)],
       [(0, 128), (128, 16)]]

Exp = mybir.ActivationFunctionType.Exp
Identity = mybir.ActivationFunctionType.Identity


def build(nc):
    xv = nc.dram_tensor("xv", [T, C, PIX], F32, kind="ExternalInput")
    wqt = nc.dram_tensor("wqt", [C, C], F32, kind="ExternalInput")
    wkt = nc.dram_tensor("wkt", [C, C], F32, kind="ExternalInput")
    wvt = nc.dram_tensor("wvt", [C, C], F32, kind="ExternalInput")
    wot = nc.dram_tensor("wot", [9, C, C], F32, kind="ExternalInput")
    bq = nc.dram_tensor("bq", [C], F32, kind="ExternalInput")
    bk = nc.dram_tensor("bk", [C], F32, kind="ExternalInput")
    bv = nc.dram_tensor("bv", [C], F32, kind="ExternalInput")
    bo = nc.dram_tensor("bo", [C], F32, kind="ExternalInput")
    out = nc.dram_tensor("out", [C, PIX], F32, kind="ExternalOutput")

    alt = [0]

    def bias_copy_alt(dst, src, bias_ap):
        alt[0] ^= 1
        if alt[0]:
            nc.scalar.activation(out=dst, in_=src, func=Identity,
                                 bias=bias_ap, scale=1.0)
        else:
            nc.vector.tensor_scalar_add(dst, src, bias_ap)

    rr = [0]

    def copy_rr(dst, src):
        # weighted: vector/scalar 2x share, gpsimd 1x (strided copies
        # run ~2x slower there, but it has idle capacity)
        rr[0] = (rr[0] + 1) % 5
        if rr[0] in (0, 2):
            nc.vector.tensor_copy(dst, src)
        elif rr[0] in (1, 3):
            nc.scalar.copy(dst, src)
        else:
            nc.gpsimd.tensor_copy(dst, src)

    sv = [0]

    def copy_sv(dst, src):
        # PSUM sources: scalar/vector only (gpsimd can't read PSUM)
        sv[0] ^= 1
        if sv[0]:
            nc.scalar.copy(dst, src)
        else:
            nc.vector.tensor_copy(dst, src)

    with tile.TileContext(nc, pool_alloc_mode="queue") as tc, ExitStack() as top:
        persist = top.enter_context(tc.tile_pool(name="persist", bufs=1))
        dramp = top.enter_context(tc.tile_pool(name="dram", bufs=1, space="DRAM"))

        # fp16 weights (cast from f32 loads)
        wq_sb, wk_sb, wv_sb = [None, None], [None, None], [None, None]
        with tc.tile_pool(name="wload", bufs=2) as p_wl:
            for name, dt_, lst in (("wq", wqt, wq_sb), ("wk", wkt, wk_sb),
                                   ("wv", wvt, wv_sb)):
                for cb in range(2):
                    tf = p_wl.tile([128, C], F32, name="wl", tag="wl")
                    nc.sync.dma_start(out=tf,
                                      in_=dt_.ap()[cb * 128:(cb + 1) * 128, :])
                    t = persist.tile([128, C], FP16, name=f"{name}{cb}",
                                     tag=f"{name}{cb}")
                    nc.vector.tensor_copy(t, tf)
                    lst[cb] = t

        def bias_tile(name, dt_):
            t = persist.tile([128, 2], F32, name=name, tag=name)
            nc.sync.dma_start(
                out=t, in_=bass.AP(tensor=dt_.ap().tensor, offset=0,
                                   ap=[[1, 128], [128, 2]]))
            return t

        bq_sb = bias_tile("bq", bq)
        bk_sb = bias_tile("bk", bk)
        bo_sb = bias_tile("bo", bo)
        bv_sb = bias_tile("bv", bv)
        ident = persist.tile([128, 128], FP16, name="ident", tag="ident")
        make_identity(nc, ident)
        zrow = persist.tile([128, 98], FP16, name="zrow", tag="zrow")
        nc.vector.memset(zrow, 0.0)

        # DRAM scratch: V in [token, d] layout per branch + att0 spill
        vdram = [dramp.tile([NKP[0], NCH[0] * 128], FP16, name="vd0", tag="vd0"),
                 dramp.tile([NKP[1], NCH[1] * 128], FP16, name="vd1", tag="vd1")]
        att0_dram = dramp.tile([128, 98 * 98], FP16, name="att0d", tag="att0d")

        # att1 outlives pass 1 (used by phase D) -> open before the passes
        esAtt1 = ExitStack()
        p_att1 = esAtt1.enter_context(tc.tile_pool(name="att1", bufs=1))
        att_sb = {}

        for br in range(2):
            psz, ohb, ntf, nch = PSZ[br], OHB[br], NTF[br], NCH[br]
            nkp, ntile = NKP[br], NTILE[br]
            nmk = 2 if br == 0 else 1
            mkw = ntf // nmk            # 288 / 144

            es_pt = ExitStack()
            p_pt = es_pt.enter_context(tc.tile_pool(name=f"pt{br}", bufs=1))
            pt_t = [p_pt.tile([128, ntf], FP16, name=f"pt{br}_{i}",
                              tag=f"pt{br}_{i}") for i in range(ntile)]
            es_P = ExitStack()
            p_P = es_P.enter_context(tc.tile_pool(name=f"P{br}", bufs=1))
            p_t = [p_P.tile([128, nkp], FP16, name=f"p{br}_{i}",
                            tag=f"p{br}_{i}") for i in range(len(NQB[br]))]
            p_run = es_P.enter_context(tc.tile_pool(name=f"run{br}", bufs=1))
            run_mx = [p_run.tile([128, 1], F32, name=f"mx{i}", tag=f"mx{i}")
                      for i in range(len(NQB[br]))]
            run_ls = [p_run.tile([128, 1], F32, name=f"ls{i}", tag=f"ls{i}")
                      for i in range(len(NQB[br]))]
            es_qw = ExitStack()
            p_qw = es_qw.enter_context(tc.tile_pool(name=f"qw{br}", bufs=1))
            qw = p_qw.tile([128, nch * ntf], FP16, name=f"qw{br}",
                           tag=f"qw{br}")
            p_stat = es_qw.enter_context(tc.tile_pool(name=f"stat{br}",
                                                      bufs=4))

            # x chunking: 3 / 2 patch-rows per chunk
            ohc = 3 if br == 0 else 2
            nql = ohb // ohc            # 8 / 6 chunks per frame
            csz = ohc * psz * W         # 1152 / 1536 pixels
            vseg = 1 if br == 0 else 2  # V staging column segments
            vsw = nch * 128 // vseg     # 2048 / 4096 cols per segment

            with tc.tile_pool(name=f"xc{br}", bufs=2) as p_xc, \
                 tc.tile_pool(name=f"xw{br}", bufs=2) as p_xw, \
                 tc.tile_pool(name=f"kw{br}", bufs=1) as p_kw, \
                 tc.tile_pool(name=f"vs{br}", bufs=2) as p_vs, \
                 tc.tile_pool(name=f"cps{br}", bufs=2, space="PSUM") as p_cps, \
                 tc.tile_pool(name=f"sps{br}", bufs=2, space="PSUM") as p_sps, \
                 tc.tile_pool(name=f"vps{br}", bufs=2, space="PSUM") as p_vps:
                for pos in range(T):
                    # ---- gather x into window-major xw (fp16) ----
                    xw = [p_xw.tile([128, nch * ntf], FP16, name=f"xw{cb}",
                                    tag=f"xw{cb}") for cb in range(2)]
                    xwv = [xw[cb].rearrange(
                        "p (wy wx oh ow) -> p wy wx oh ow",
                        wy=psz, wx=psz, oh=ohb, ow=ohb) for cb in range(2)]
                    for chq in range(nql):
                        for cb in range(2):
                            xc = p_xc.tile([128, csz], F32, name=f"xc{cb}",
                                           tag=f"xc{cb}")
                            nc.sync.dma_start(
                                out=xc,
                                in_=xv.ap()[pos, cb * 128:(cb + 1) * 128,
                                            chq * csz:(chq + 1) * csz])
                            xcv = xc.rearrange(
                                "p (oh hh ow ww) -> p oh hh ow ww",
                                oh=ohc, hh=psz, ow=ohb, ww=psz)
                            for wy in range(psz):
                                src = xcv[:, :, wy, :, :].rearrange(
                                    "p a b c -> p c a b")
                                dst = xwv[cb][:, wy, :,
                                              chq * ohc:(chq + 1) * ohc, :]
                                copy_rr(dst, src)

                    # ---- Q conv (own frame only) and K conv from xw ----
                    def conv_from_xw(w_sb, b_sb, dst_tile):
                        for k in range(nch * ntf // 512):
                            ps = p_cps.tile([128, 512], F32, name="cps",
                                            tag="cps")
                            for cb in range(2):
                                nc.tensor.matmul(
                                    ps,
                                    w_sb[cb][:, br * 128:(br + 1) * 128],
                                    xw[cb][:, k * 512:(k + 1) * 512],
                                    start=(cb == 0), stop=(cb == 1))
                            bias_copy_alt(dst_tile[:, k * 512:(k + 1) * 512],
                                          ps, b_sb[:, br:br + 1])

                    if pos == 0:
                        conv_from_xw(wq_sb, bq_sb, qw)
                    kw = p_kw.tile([128, nch * ntf], FP16, name="kw", tag="kw")
                    conv_from_xw(wk_sb, bk_sb, kw)

                    # ---- scores + online softmax ----
                    for nqi, (q0, nqsz) in enumerate(NQB[br]):
                        pss = [p_sps.tile([128, mkw], F32, name=f"s{mkh}",
                                          tag=f"s{mkh}")
                               for mkh in range(nmk)]
                        for ci in range(nch):
                            for mkh in range(nmk):
                                nc.tensor.matmul(
                                    pss[mkh][:nqsz],
                                    qw[:, ci * ntf + q0:ci * ntf + q0 + nqsz],
                                    kw[:, ci * ntf + mkh * mkw:
                                       ci * ntf + (mkh + 1) * mkw],
                                    start=(ci == 0), stop=(ci == nch - 1))
                        for mkh in range(nmk):
                            ps = pss[mkh]
                            o = pos * ntf + mkh * mkw
                            pt = p_t[nqi]
                            mx, ls = run_mx[nqi], run_ls[nqi]
                            bm = p_stat.tile([128, 1], F32, name="bm",
                                             tag="bm")
                            nc.vector.reduce_max(out=bm[:nqsz],
                                                 in_=ps[:nqsz, :],
                                                 axis=mybir.AxisListType.X)
                            if pos == 0 and mkh == 0:
                                nc.vector.tensor_copy(mx[:nqsz], bm[:nqsz])
                                nmx = p_stat.tile([128, 1], F32, name="nmx",
                                                  tag="nmx")
                                nc.vector.tensor_scalar_mul(
                                    nmx[:nqsz], mx[:nqsz], -SC[br])
                                nc.scalar.activation(
                                    out=pt[:nqsz, o:o + mkw],
                                    in_=ps[:nqsz, :], func=Exp,
                                    bias=nmx[:nqsz], scale=SC[br],
                                    accum_out=ls[:nqsz])
                            else:
                                nmax = p_stat.tile([128, 1], F32,
                                                   name="nmax", tag="nmax")
                                nc.vector.tensor_max(nmax[:nqsz], mx[:nqsz],
                                                     bm[:nqsz])
                                nmx = p_stat.tile([128, 1], F32, name="nmx",
                                                  tag="nmx")
                                nc.vector.tensor_scalar_mul(
                                    nmx[:nqsz], nmax[:nqsz], -SC[br])
                                delta = p_stat.tile([128, 1], F32,
                                                    name="delta", tag="delta")
                                nc.scalar.activation(
                                    out=delta[:nqsz], in_=mx[:nqsz],
                                    func=Exp, bias=nmx[:nqsz], scale=SC[br])
                                nc.vector.tensor_scalar_mul(
                                    pt[:nqsz, 0:o], pt[:nqsz, 0:o],
                                    delta[:nqsz])
                                pl = p_stat.tile([128, 1], F32, name="pl",
                                                 tag="pl")
                                nc.scalar.activation(
                                    out=pt[:nqsz, o:o + mkw],
                                    in_=ps[:nqsz, :], func=Exp,
                                    bias=nmx[:nqsz], scale=SC[br],
                                    accum_out=pl[:nqsz])
                                nc.vector.scalar_tensor_tensor(
                                    out=ls[:nqsz], in0=ls[:nqsz],
                                    scalar=delta[:nqsz], in1=pl[:nqsz],
                                    op0=mybir.AluOpType.mult,
                                    op1=mybir.AluOpType.add)
                                nc.vector.tensor_copy(mx[:nqsz], nmax[:nqsz])

                    # ---- V conv from xw -> DRAM scratch ----
                    # psum batches 4 ci-chunks per bank -> 4x fewer copies
                    for gi, (g0, gm) in enumerate(NQB[br]):
                        for seg in range(vseg):
                            vstage = p_vs.tile([128, vsw], FP16,
                                               name="vs", tag="vs")
                            nseg = nch // vseg
                            for cq in range(nseg // 4):
                                ps = p_vps.tile([128, 512], F32, name="vps",
                                                tag="vps")
                                for cis4 in range(4):
                                    cis = cq * 4 + cis4
                                    ci = seg * nseg + cis
                                    for cb in range(2):
                                        nc.tensor.matmul(
                                            ps[:gm, cis4 * 128:
                                               (cis4 + 1) * 128],
                                            xw[cb][:, ci * ntf + g0:
                                                   ci * ntf + g0 + gm],
                                            wv_sb[cb][:,
                                                      br * 128:(br + 1) * 128],
                                            start=(cb == 0), stop=(cb == 1))
                                copy_sv(vstage[:gm, cq * 512:(cq + 1) * 512],
                                        ps[:gm, :])
                            row0 = pos * ntf + g0
                            nc.gpsimd.dma_start(
                                out=vdram[br][row0:row0 + gm,
                                              seg * vsw:(seg + 1) * vsw],
                                in_=vstage[:gm, :])

                # ---- finalize softmax: P /= ls ----
                for nqi, (q0, nqsz) in enumerate(NQB[br]):
                    rs = p_stat.tile([128, 1], F32, name="rs", tag="rs")
                    nc.vector.reciprocal(rs[:nqsz], run_ls[nqi][:nqsz])
                    nc.vector.tensor_scalar_mul(
                        p_t[nqi][:nqsz, :], p_t[nqi][:nqsz, :], rs[:nqsz])
            es_qw.close()

            # ---- P^T transposes ----
            with tc.tile_pool(name=f"tp{br}", bufs=2, space="PSUM") as p_tp:
                for ti in range(ntile):
                    t0 = ti * 128
                    m = min(128, nkp - t0)
                    for nqi, (q0, nqsz) in enumerate(NQB[br]):
                        tp = p_tp.tile([128, 128], FP16, name="tp", tag="tp")
                        nc.tensor.transpose(
                            tp[:m, :nqsz], p_t[nqi][:nqsz, t0:t0 + m],
                            ident[:nqsz, :nqsz])
                        copy_sv(pt_t[ti][:m, q0:q0 + nqsz], tp[:m, :nqsz])
            es_P.close()

            # ---- PV: y^T accumulated over key tiles -> att ----
            es_att0 = ExitStack()
            if br == 0:
                p_att = es_att0.enter_context(
                    tc.tile_pool(name="att0", bufs=1))
            else:
                p_att = p_att1
            att = p_att.tile([128, 98 * 98], FP16, name=f"att{br}",
                             tag=f"att{br}")
            att_sb[br] = att
            attv = att.rearrange("p (h w) -> p h w", h=98)
            nc.scalar.copy(att[:, 0:98], zrow)
            nc.scalar.copy(att[:, 97 * 98:98 * 98], zrow)
            zcol = zrow[:, 0:96].rearrange("p (a c) -> p a c", a=96)
            nc.vector.tensor_copy(attv[:, 1:97, 0:1], zcol)
            nc.vector.tensor_copy(attv[:, 1:97, 97:98], zcol)
            wvw = attv[:, 1:97, 1:97].rearrange(
                "p (oh hh) (ow ww) -> p oh hh ow ww", hh=psz, ww=psz)

            es_vt = ExitStack()
            p_vt = es_vt.enter_context(tc.tile_pool(name=f"vt{br}", bufs=1))
            vt = []
            for ti in range(ntile):
                t0 = ti * 128
                m = min(128, nkp - t0)
                t = p_vt.tile([128, nch * 128], FP16, name=f"vt{ti}",
                              tag=f"vt{ti}")
                nc.gpsimd.dma_start(out=t[:m, :], in_=vdram[br][t0:t0 + m, :])
                vt.append(t)

            nqh_n = 2 if br == 0 else 1
            nqw = ntf // nqh_n
            ohq = ohb // nqh_n
            with tc.tile_pool(name=f"pv{br}", bufs=2, space="PSUM") as p_pv:
                for ci in range(nch):
                    wy, wx = divmod(ci, psz)
                    # ti outer / nqh inner: stationary vt chunk reused
                    # across both query halves (one LDWEIGHTS, 2 matmuls)
                    pss = [p_pv.tile([128, nqw], F32, name=f"pv{nqh}",
                                     tag=f"pv{nqh}")
                           for nqh in range(nqh_n)]
                    for ti in range(ntile):
                        m = min(128, nkp - ti * 128)
                        for nqh in range(nqh_n):
                            nc.tensor.matmul(
                                pss[nqh],
                                vt[ti][:m, ci * 128:(ci + 1) * 128],
                                pt_t[ti][:m, nqh * nqw:(nqh + 1) * nqw],
                                start=(ti == 0), stop=(ti == ntile - 1))
                    for nqh in range(nqh_n):
                        dst = wvw[:, nqh * ohq:(nqh + 1) * ohq, wy, :, wx]
                        src = pss[nqh].rearrange("p (a c) -> p a c", a=ohq)
                        bias_copy_alt(dst, src, bv_sb[:, br:br + 1])
            es_vt.close()
            if br == 0:
                nc.gpsimd.dma_start(out=att0_dram, in_=att)
                es_att0.close()
            es_pt.close()

        # ---------------- phase D: 3x3 conv + LeakyReLU ----------------
        with tc.tile_pool(name="attr", bufs=1) as p_attr, \
             tc.tile_pool(name="wot", bufs=1) as p_wot, \
             tc.tile_pool(name="wotl", bufs=2) as p_wotl, \
             tc.tile_pool(name="dout", bufs=3) as p_do, \
             tc.tile_pool(name="dps", bufs=4, space="PSUM") as p_dps:
            att0 = p_attr.tile([128, 98 * 98], FP16, name="attr0",
                               tag="attr0")
            nc.sync.dma_start(out=att0, in_=att0_dram)
            att_in = [att0, att_sb[1]]
            wot_sb = []
            for cb in range(2):
                tf = p_wotl.tile([128, 9 * C], F32, name="wotl", tag="wotl")
                nc.sync.dma_start(
                    out=tf.rearrange("i (t o) -> i t o", t=9),
                    in_=wot.ap()[:, cb * 128:(cb + 1) * 128, :].rearrange(
                        "t i o -> i t o"))
                t = p_wot.tile([128, 9, C], FP16, name=f"wot{cb}",
                               tag=f"wot{cb}")
                nc.vector.tensor_copy(t, tf.rearrange("i (t o) -> i t o",
                                                      t=9))
                wot_sb.append(t)
            attv2 = [att_in[cb].rearrange("p (h w) -> p h w", h=98)
                     for cb in range(2)]
            for coutb in range(2):
                for rg in range(24):
                    ps = p_dps.tile([128, 384], F32, name="dps", tag="dps")
                    k = 0
                    for cb in range(2):
                        for tap in range(9):
                            dy, dx = divmod(tap, 3)
                            rhs = attv2[cb][:, rg * 4 + dy:rg * 4 + dy + 4,
                                            dx:dx + 96]
                            lhsT = wot_sb[cb][:, tap,
                                              coutb * 128:(coutb + 1) * 128]
                            nc.tensor.matmul(ps, lhsT, rhs,
                                             start=(k == 0), stop=(k == 17))
                            k += 1
                    t1 = p_do.tile([128, 384], F32, name="t1", tag="t1")
                    nc.scalar.activation(out=t1, in_=ps, func=Identity,
                                         bias=bo_sb[:, coutb:coutb + 1],
                                         scale=1.0)
                    t2 = p_do.tile([128, 384], F32, name="t2", tag="t2")
                    nc.vector.scalar_tensor_tensor(
                        out=t2, in0=t1, scalar=0.2, in1=t1,
                        op0=mybir.AluOpType.mult, op1=mybir.AluOpType.max)
                    nc.sync.dma_start(
                        out=out.ap()[coutb * 128:(coutb + 1) * 128,
                                     rg * 384:(rg + 1) * 384],
                        in_=t2)
        esAtt1.close()
    return nc


_CACHED = {}


def _get_nc():
    if "nc" not in _CACHED:
        nc = bacc.Bacc("TRN2", debug=False, target_bir_lowering=False)
        build(nc)
        nc.compile()
        _CACHED["nc"] = nc
    return _CACHED["nc"]


def make_in_maps(x, wq, bq_, wk, bk_, wv, bv_, wo, bo_):
    shared = {
        "wqt": np.ascontiguousarray(wq.T.astype(np.float32)),
        "wkt": np.ascontiguousarray(wk.T.astype(np.float32)),
        "wvt": np.ascontiguousarray(wv.T.astype(np.float32)),
        "wot": np.ascontiguousarray(
            wo.transpose(2, 3, 1, 0).reshape(9, C, C).astype(np.float32)),
        "bq": np.ascontiguousarray(bq_.astype(np.float32)),
        "bk": np.ascontiguousarray(bk_.astype(np.float32)),
        "bv": np.ascontiguousarray(bv_.astype(np.float32)),
        "bo": np.ascontiguousarray(bo_.astype(np.float32)),
    }
    x3 = np.ascontiguousarray(x.reshape(2 * T, C, PIX).astype(np.float32))
    in_maps = []
    for core in range(NCORES):
        v, f = divmod(core, T)
        perm = [f] + [g for g in range(T) if g != f]
        m = dict(shared)
        m["xv"] = np.ascontiguousarray(x3[v * T:(v + 1) * T][perm])
        in_maps.append(m)
    return in_maps


def kernel(**inputs):
    from concourse.bass_utils import run_bass_kernel_spmd

    x = np.asarray(inputs["x"], dtype=np.float32)
    in_maps = make_in_maps(
        x, np.asarray(inputs["wq"]), np.asarray(inputs["bq"]),
        np.asarray(inputs["wk"]), np.asarray(inputs["bk"]),
        np.asarray(inputs["wv"]), np.asarray(inputs["bv"]),
        np.asarray(inputs["wo"]), np.asarray(inputs["bo"]))
    nc = _get_nc()
    res = run_bass_kernel_spmd(nc, in_maps, core_ids=list(range(NCORES)))
    outs = [res.results[c]["out"].reshape(C, H, W) for c in range(NCORES)]
    return np.stack(outs).astype(np.float32)


# revision 27
# speedup vs baseline: 1.1909x; 1.0106x over previous
"""Trainium2 Bass kernel for nn_MultiHeadedAttention_6416681140387.

Two-branch windowed video attention:
  x [8,256,96,96] -> 1x1 conv Q/K/V -> per-branch full attention over
  window-token features (branch0: 4x4 patches, d=2048, 2304 key tokens;
  branch1: 8x8 patches, d=8192, 576 key tokens) -> concat channels
  -> 3x3 conv + LeakyReLU(0.2).

Sharding: 8 cores = (video b in {0,1}) x (frame t in {0..3}). Each core
computes its full output frame [256,96,96]. The host permutes each core's
4-frame video slice so the core's own frame comes first; attention is
key-order invariant so P columns / V rows just follow processing order.

Design (gather-once, window-major convs):
  Per branch pass, per frame: x is gathered once into window-major xw
  (fp16), and Q (frame 0 only) / K / V 1x1-convs all consume xw. K conv
  output is therefore window-major => scores matmuls get contiguous rhs.
  V tiles ([token, d] layout) are spilled to a DRAM scratch and streamed
  back for the PV phase (all-SBUF working set stays under the 192KB cap).
  Whole 16-bit path is fp16 (better mantissa than bf16, same PE rate).
"""

import sys

if "/opt/trn_rl_repo" not in sys.path:
    sys.path.insert(0, "/opt/trn_rl_repo")

import math
from contextlib import ExitStack

import numpy as np

import concourse.bass as bass
import concourse.tile as tile
from concourse import bacc, mybir
from concourse.masks import make_identity

F32 = mybir.dt.float32
FP16 = mybir.dt.float16

T = 4
C = 256
H = W = 96
PIX = H * W
NCORES = 8

PSZ = [4, 8]
OHB = [24, 12]                  # token grid side per branch
NTF = [576, 144]                # tokens per frame
NKP = [2304, 576]               # key tokens per video (no padding)
NCH = [16, 64]                  # d-chunks (psz^2)
NTILE = [18, 5]                 # ceil(NKP/128)
SC = [1.0 / math.sqrt(2048.0), 1.0 / math.sqrt(8192.0)]
NQB = [[(0, 128), (128, 128), (256, 128), (384, 128), (512, 64)],
       [(0, 128), (128, 16)]]

Exp = mybir.ActivationFunctionType.Exp
Identity = mybir.ActivationFunctionType.Identity


def build(nc):
    xv = nc.dram_tensor("xv", [T, C, PIX], F32, kind="ExternalInput")
    wqt = nc.dram_tensor("wqt", [C, C], F32, kind="ExternalInput")
    wkt = nc.dram_tensor("wkt", [C, C], F32, kind="ExternalInput")
    wvt = nc.dram_tensor("wvt", [C, C], F32, kind="ExternalInput")
    wot = nc.dram_tensor("wot", [9, C, C], F32, kind="ExternalInput")
    bq = nc.dram_tensor("bq", [C], F32, kind="ExternalInput")
    bk = nc.dram_tensor("bk", [C], F32, kind="ExternalInput")
    bv = nc.dram_tensor("bv", [C], F32, kind="ExternalInput")
    bo = nc.dram_tensor("bo", [C], F32, kind="ExternalInput")
    out = nc.dram_tensor("out", [C, PIX], F32, kind="ExternalOutput")

    alt = [0]

    def bias_copy_alt(dst, src, bias_ap):
        alt[0] ^= 1
        if alt[0]:
            nc.scalar.activation(out=dst, in_=src, func=Identity,
                                 bias=bias_ap, scale=1.0)
        else:
            nc.vector.tensor_scalar_add(dst, src, bias_ap)

    rr = [0]

    def copy_rr(dst, src):
        # weighted: vector/scalar 2x share, gpsimd 1x (strided copies
        # run ~2x slower there, but it has idle capacity)
        rr[0] = (rr[0] + 1) % 5
        if rr[0] in (0, 2):
            nc.vector.tensor_copy(dst, src)
        elif rr[0] in (1, 3):
            nc.scalar.copy(dst, src)
        else:
            nc.gpsimd.tensor_copy(dst, src)

    sv = [0]

    def copy_sv(dst, src):
        # PSUM sources: scalar/vector only (gpsimd can't read PSUM)
        sv[0] ^= 1
        if sv[0]:
            nc.scalar.copy(dst, src)
        else:
            nc.vector.tensor_copy(dst, src)

    with tile.TileContext(nc, pool_alloc_mode="queue") as tc, ExitStack() as top:
        persist = top.enter_context(tc.tile_pool(name="persist", bufs=1))
        dramp = top.enter_context(tc.tile_pool(name="dram", bufs=1, space="DRAM"))

        # fp16 weights (cast from f32 loads)
        wq_sb, wk_sb, wv_sb = [None, None], [None, None], [None, None]
        with tc.tile_pool(name="wload", bufs=2) as p_wl:
            for name, dt_, lst in (("wq", wqt, wq_sb), ("wk", wkt, wk_sb),
                                   ("wv", wvt, wv_sb)):
                for cb in range(2):
                    tf = p_wl.tile([128, C], F32, name="wl", tag="wl")
                    nc.sync.dma_start(out=tf,
                                      in_=dt_.ap()[cb * 128:(cb + 1) * 128, :])
                    t = persist.tile([128, C], FP16, name=f"{name}{cb}",
                                     tag=f"{name}{cb}")
                    nc.vector.tensor_copy(t, tf)
                    lst[cb] = t

        def bias_tile(name, dt_):
            t = persist.tile([128, 2], F32, name=name, tag=name)
            nc.sync.dma_start(
                out=t, in_=bass.AP(tensor=dt_.ap().tensor, offset=0,
                                   ap=[[1, 128], [128, 2]]))
            return t

        bq_sb = bias_tile("bq", bq)
        bk_sb = bias_tile("bk", bk)
        bo_sb = bias_tile("bo", bo)
        bv_sb = bias_tile("bv", bv)
        ident = persist.tile([128, 128], FP16, name="ident", tag="ident")
        make_identity(nc, ident)
        zrow = persist.tile([128, 98], FP16, name="zrow", tag="zrow")
        nc.vector.memset(zrow, 0.0)

        # DRAM scratch: V in [token, d] layout per branch + att0 spill
        vdram = [dramp.tile([NKP[0], NCH[0] * 128], FP16, name="vd0", tag="vd0"),
                 dramp.tile([NKP[1], NCH[1] * 128], FP16, name="vd1", tag="vd1")]
        att0_dram = dramp.tile([128, 98 * 98], FP16, name="att0d", tag="att0d")

        # att1 outlives pass 1 (used by phase D) -> open before the passes
        esAtt1 = ExitStack()
        p_att1 = esAtt1.enter_context(tc.tile_pool(name="att1", bufs=1))
        att_sb = {}

        for br in range(2):
            psz, ohb, ntf, nch = PSZ[br], OHB[br], NTF[br], NCH[br]
            nkp, ntile = NKP[br], NTILE[br]
            nmk = 2 if br == 0 else 1
            mkw = ntf // nmk            # 288 / 144

            es_pt = ExitStack()
            p_pt = es_pt.enter_context(tc.tile_pool(name=f"pt{br}", bufs=1))
            pt_t = [p_pt.tile([128, ntf], FP16, name=f"pt{br}_{i}",
                              tag=f"pt{br}_{i}") for i in range(ntile)]
            es_P = ExitStack()
            p_P = es_P.enter_context(tc.tile_pool(name=f"P{br}", bufs=1))
            p_t = [p_P.tile([128, nkp], FP16, name=f"p{br}_{i}",
                            tag=f"p{br}_{i}") for i in range(len(NQB[br]))]
            p_run = es_P.enter_context(tc.tile_pool(name=f"run{br}", bufs=1))
            run_mx = [p_run.tile([128, 1], F32, name=f"mx{i}", tag=f"mx{i}")
                      for i in range(len(NQB[br]))]
            run_ls = [p_run.tile([128, 1], F32, name=f"ls{i}", tag=f"ls{i}")
                      for i in range(len(NQB[br]))]
            es_qw = ExitStack()
            p_qw = es_qw.enter_context(tc.tile_pool(name=f"qw{br}", bufs=1))
            qw = p_qw.tile([128, nch * ntf], FP16, name=f"qw{br}",
                           tag=f"qw{br}")
            p_stat = es_qw.enter_context(tc.tile_pool(name=f"stat{br}",
                                                      bufs=4))

            # x chunking: 3 / 2 patch-rows per chunk
            ohc = 3 if br == 0 else 2
            nql = ohb // ohc            # 8 / 6 chunks per frame
            csz = ohc * psz * W         # 1152 / 1536 pixels
            vseg = 1 if br == 0 else 2  # V staging column segments
            vsw = nch * 128 // vseg     # 2048 / 4096 cols per segment

            with tc.tile_pool(name=f"xc{br}", bufs=2) as p_xc, \
                 tc.tile_pool(name=f"xw{br}", bufs=2) as p_xw, \
                 tc.tile_pool(name=f"kw{br}", bufs=1 if br == 0 else 2) as p_kw, \
                 tc.tile_pool(name=f"vs{br}", bufs=2) as p_vs, \
                 tc.tile_pool(name=f"cps{br}", bufs=2, space="PSUM") as p_cps, \
                 tc.tile_pool(name=f"sps{br}", bufs=2, space="PSUM") as p_sps, \
                 tc.tile_pool(name=f"vps{br}", bufs=2, space="PSUM") as p_vps:
                for pos in range(T):
                    # ---- gather x into window-major xw (fp16) ----
                    xw = [p_xw.tile([128, nch * ntf], FP16, name=f"xw{cb}",
                                    tag=f"xw{cb}") for cb in range(2)]
                    xwv = [xw[cb].rearrange(
                        "p (wy wx oh ow) -> p wy wx oh ow",
                        wy=psz, wx=psz, oh=ohb, ow=ohb) for cb in range(2)]
                    for chq in range(nql):
                        for cb in range(2):
                            xc = p_xc.tile([128, csz], F32, name=f"xc{cb}",
                                           tag=f"xc{cb}")
                            nc.sync.dma_start(
                                out=xc,
                                in_=xv.ap()[pos, cb * 128:(cb + 1) * 128,
                                            chq * csz:(chq + 1) * csz])
                            xcv = xc.rearrange(
                                "p (oh hh ow ww) -> p oh hh ow ww",
                                oh=ohc, hh=psz, ow=ohb, ww=psz)
                            for wy in range(psz):
                                src = xcv[:, :, wy, :, :].rearrange(
                                    "p a b c -> p c a b")
                                dst = xwv[cb][:, wy, :,
                                              chq * ohc:(chq + 1) * ohc, :]
                                copy_rr(dst, src)

                    # ---- Q conv (own frame only) and K conv from xw ----
                    def conv_from_xw(w_sb, b_sb, dst_tile):
                        for k in range(nch * ntf // 512):
                            ps = p_cps.tile([128, 512], F32, name="cps",
                                            tag="cps")
                            for cb in range(2):
                                nc.tensor.matmul(
                                    ps,
                                    w_sb[cb][:, br * 128:(br + 1) * 128],
                                    xw[cb][:, k * 512:(k + 1) * 512],
                                    start=(cb == 0), stop=(cb == 1))
                            bias_copy_alt(dst_tile[:, k * 512:(k + 1) * 512],
                                          ps, b_sb[:, br:br + 1])

                    if pos == 0:
                        conv_from_xw(wq_sb, bq_sb, qw)
                    kw = p_kw.tile([128, nch * ntf], FP16, name="kw", tag="kw")
                    conv_from_xw(wk_sb, bk_sb, kw)

                    # ---- scores + online softmax ----
                    for nqi, (q0, nqsz) in enumerate(NQB[br]):
                        pss = [p_sps.tile([128, mkw], F32, name=f"s{mkh}",
                                          tag=f"s{mkh}")
                               for mkh in range(nmk)]
                        for ci in range(nch):
                            for mkh in range(nmk):
                                nc.tensor.matmul(
                                    pss[mkh][:nqsz],
                                    qw[:, ci * ntf + q0:ci * ntf + q0 + nqsz],
                                    kw[:, ci * ntf + mkh * mkw:
                                       ci * ntf + (mkh + 1) * mkw],
                                    start=(ci == 0), stop=(ci == nch - 1))
                        for mkh in range(nmk):
                            ps = pss[mkh]
                            o = pos * ntf + mkh * mkw
                            pt = p_t[nqi]
                            mx, ls = run_mx[nqi], run_ls[nqi]
                            bm = p_stat.tile([128, 1], F32, name="bm",
                                             tag="bm")
                            nc.vector.reduce_max(out=bm[:nqsz],
                                                 in_=ps[:nqsz, :],
                                                 axis=mybir.AxisListType.X)
                            if pos == 0 and mkh == 0:
                                nc.vector.tensor_copy(mx[:nqsz], bm[:nqsz])
                                nmx = p_stat.tile([128, 1], F32, name="nmx",
                                                  tag="nmx")
                                nc.vector.tensor_scalar_mul(
                                    nmx[:nqsz], mx[:nqsz], -SC[br])
                                nc.scalar.activation(
                                    out=pt[:nqsz, o:o + mkw],
                                    in_=ps[:nqsz, :], func=Exp,
                                    bias=nmx[:nqsz], scale=SC[br],
                                    accum_out=ls[:nqsz])
                            else:
                                nmax = p_stat.tile([128, 1], F32,
                                                   name="nmax", tag="nmax")
                                nc.vector.tensor_max(nmax[:nqsz], mx[:nqsz],
                                                     bm[:nqsz])
                                nmx = p_stat.tile([128, 1], F32, name="nmx",
                                                  tag="nmx")
                                nc.vector.tensor_scalar_mul(
                                    nmx[:nqsz], nmax[:nqsz], -SC[br])
                                delta = p_stat.tile([128, 1], F32,
                                                    name="delta", tag="delta")
                                nc.scalar.activation(
                                    out=delta[:nqsz], in_=mx[:nqsz],
                                    func=Exp, bias=nmx[:nqsz], scale=SC[br])
                                nc.vector.tensor_scalar_mul(
                                    pt[:nqsz, 0:o], pt[:nqsz, 0:o],
                                    delta[:nqsz])
                                pl = p_stat.tile([128, 1], F32, name="pl",
                                                 tag="pl")
                                nc.scalar.activation(
                                    out=pt[:nqsz, o:o + mkw],
                                    in_=ps[:nqsz, :], func=Exp,
                                    bias=nmx[:nqsz], scale=SC[br],
                                    accum_out=pl[:nqsz])
                                nc.vector.scalar_tensor_tensor(
                                    out=ls[:nqsz], in0=ls[:nqsz],
                                    scalar=delta[:nqsz], in1=pl[:nqsz],
                                    op0=mybir.AluOpType.mult,
                                    op1=mybir.AluOpType.add)
                                nc.vector.tensor_copy(mx[:nqsz], nmax[:nqsz])

                    # ---- V conv from xw -> DRAM scratch ----
                    # psum batches 4 ci-chunks per bank -> 4x fewer copies
                    for gi, (g0, gm) in enumerate(NQB[br]):
                        for seg in range(vseg):
                            vstage = p_vs.tile([128, vsw], FP16,
                                               name="vs", tag="vs")
                            nseg = nch // vseg
                            for cq in range(nseg // 4):
                                ps = p_vps.tile([128, 512], F32, name="vps",
                                                tag="vps")
                                for cis4 in range(4):
                                    cis = cq * 4 + cis4
                                    ci = seg * nseg + cis
                                    for cb in range(2):
                                        nc.tensor.matmul(
                                            ps[:gm, cis4 * 128:
                                               (cis4 + 1) * 128],
                                            xw[cb][:, ci * ntf + g0:
                                                   ci * ntf + g0 + gm],
                                            wv_sb[cb][:,
                                                      br * 128:(br + 1) * 128],
                                            start=(cb == 0), stop=(cb == 1))
                                copy_sv(vstage[:gm, cq * 512:(cq + 1) * 512],
                                        ps[:gm, :])
                            row0 = pos * ntf + g0
                            nc.gpsimd.dma_start(
                                out=vdram[br][row0:row0 + gm,
                                              seg * vsw:(seg + 1) * vsw],
                                in_=vstage[:gm, :])

                # ---- finalize softmax: P /= ls ----
                for nqi, (q0, nqsz) in enumerate(NQB[br]):
                    rs = p_stat.tile([128, 1], F32, name="rs", tag="rs")
                    nc.vector.reciprocal(rs[:nqsz], run_ls[nqi][:nqsz])
                    nc.vector.tensor_scalar_mul(
                        p_t[nqi][:nqsz, :], p_t[nqi][:nqsz, :], rs[:nqsz])
            es_qw.close()

            # ---- P^T transposes ----
            with tc.tile_pool(name=f"tp{br}", bufs=2, space="PSUM") as p_tp:
                for ti in range(ntile):
                    t0 = ti * 128
                    m = min(128, nkp - t0)
                    for nqi, (q0, nqsz) in enumerate(NQB[br]):
                        tp = p_tp.tile([128, 128], FP16, name="tp", tag="tp")
                        nc.tensor.transpose(
                            tp[:m, :nqsz], p_t[nqi][:nqsz, t0:t0 + m],
                            ident[:nqsz, :nqsz])
                        copy_sv(pt_t[ti][:m, q0:q0 + nqsz], tp[:m, :nqsz])
            es_P.close()

            # ---- PV: y^T accumulated over key tiles -> att ----
            es_att0 = ExitStack()
            if br == 0:
                p_att = es_att0.enter_context(
                    tc.tile_pool(name="att0", bufs=1))
            else:
                p_att = p_att1
            att = p_att.tile([128, 98 * 98], FP16, name=f"att{br}",
                             tag=f"att{br}")
            att_sb[br] = att
            attv = att.rearrange("p (h w) -> p h w", h=98)
            nc.scalar.copy(att[:, 0:98], zrow)
            nc.scalar.copy(att[:, 97 * 98:98 * 98], zrow)
            zcol = zrow[:, 0:96].rearrange("p (a c) -> p a c", a=96)
            nc.vector.tensor_copy(attv[:, 1:97, 0:1], zcol)
            nc.vector.tensor_copy(attv[:, 1:97, 97:98], zcol)
            wvw = attv[:, 1:97, 1:97].rearrange(
                "p (oh hh) (ow ww) -> p oh hh ow ww", hh=psz, ww=psz)

            es_vt = ExitStack()
            p_vt = es_vt.enter_context(tc.tile_pool(name=f"vt{br}", bufs=1))
            vt = []
            for ti in range(ntile):
                t0 = ti * 128
                m = min(128, nkp - t0)
                t = p_vt.tile([128, nch * 128], FP16, name=f"vt{ti}",
                              tag=f"vt{ti}")
                nc.gpsimd.dma_start(out=t[:m, :], in_=vdram[br][t0:t0 + m, :])
                vt.append(t)

            nqh_n = 2 if br == 0 else 1
            nqw = ntf // nqh_n
            ohq = ohb // nqh_n
            with tc.tile_pool(name=f"pv{br}", bufs=2, space="PSUM") as p_pv:
                for ci in range(nch):
                    wy, wx = divmod(ci, psz)
                    # ti outer / nqh inner: stationary vt chunk reused
                    # across both query halves (one LDWEIGHTS, 2 matmuls)
                    pss = [p_pv.tile([128, nqw], F32, name=f"pv{nqh}",
                                     tag=f"pv{nqh}")
                           for nqh in range(nqh_n)]
                    for ti in range(ntile):
                        m = min(128, nkp - ti * 128)
                        for nqh in range(nqh_n):
                            nc.tensor.matmul(
                                pss[nqh],
                                vt[ti][:m, ci * 128:(ci + 1) * 128],
                                pt_t[ti][:m, nqh * nqw:(nqh + 1) * nqw],
                                start=(ti == 0), stop=(ti == ntile - 1))
                    for nqh in range(nqh_n):
                        dst = wvw[:, nqh * ohq:(nqh + 1) * ohq, wy, :, wx]
                        src = pss[nqh].rearrange("p (a c) -> p a c", a=ohq)
                        bias_copy_alt(dst, src, bv_sb[:, br:br + 1])
            es_vt.close()
            if br == 0:
                nc.gpsimd.dma_start(out=att0_dram, in_=att)
                es_att0.close()
            es_pt.close()

        # ---------------- phase D: 3x3 conv + LeakyReLU ----------------
        with tc.tile_pool(name="attr", bufs=1) as p_attr, \
             tc.tile_pool(name="wot", bufs=1) as p_wot, \
             tc.tile_pool(name="wotl", bufs=2) as p_wotl, \
             tc.tile_pool(name="dout", bufs=3) as p_do, \
             tc.tile_pool(name="dps", bufs=4, space="PSUM") as p_dps:
            att0 = p_attr.tile([128, 98 * 98], FP16, name="attr0",
                               tag="attr0")
            nc.sync.dma_start(out=att0, in_=att0_dram)
            att_in = [att0, att_sb[1]]
            wot_sb = []
            for cb in range(2):
                tf = p_wotl.tile([128, 9 * C], F32, name="wotl", tag="wotl")
                nc.sync.dma_start(
                    out=tf.rearrange("i (t o) -> i t o", t=9),
                    in_=wot.ap()[:, cb * 128:(cb + 1) * 128, :].rearrange(
                        "t i o -> i t o"))
                t = p_wot.tile([128, 9, C], FP16, name=f"wot{cb}",
                               tag=f"wot{cb}")
                nc.vector.tensor_copy(t, tf.rearrange("i (t o) -> i t o",
                                                      t=9))
                wot_sb.append(t)
            attv2 = [att_in[cb].rearrange("p (h w) -> p h w", h=98)
                     for cb in range(2)]
            for coutb in range(2):
                for rg in range(24):
                    ps = p_dps.tile([128, 384], F32, name="dps", tag="dps")
                    k = 0
                    for cb in range(2):
                        for tap in range(9):
                            dy, dx = divmod(tap, 3)
                            rhs = attv2[cb][:, rg * 4 + dy:rg * 4 + dy + 4,
                                            dx:dx + 96]
                            lhsT = wot_sb[cb][:, tap,
                                              coutb * 128:(coutb + 1) * 128]
                            nc.tensor.matmul(ps, lhsT, rhs,
                                             start=(k == 0), stop=(k == 17))
                            k += 1
                    t1 = p_do.tile([128, 384], F32, name="t1", tag="t1")
                    nc.scalar.activation(out=t1, in_=ps, func=Identity,
                                         bias=bo_sb[:, coutb:coutb + 1],
                                         scale=1.0)
                    t2 = p_do.tile([128, 384], F32, name="t2", tag="t2")
                    nc.vector.scalar_tensor_tensor(
                        out=t2, in0=t1, scalar=0.2, in1=t1,
                        op0=mybir.AluOpType.mult, op1=mybir.AluOpType.max)
                    nc.sync.dma_start(
                        out=out.ap()[coutb * 128:(coutb + 1) * 128,
                                     rg * 384:(rg + 1) * 384],
                        in_=t2)
        esAtt1.close()
    return nc


_CACHED = {}


def _get_nc():
    if "nc" not in _CACHED:
        nc = bacc.Bacc("TRN2", debug=False, target_bir_lowering=False)
        build(nc)
        nc.compile()
        _CACHED["nc"] = nc
    return _CACHED["nc"]


def make_in_maps(x, wq, bq_, wk, bk_, wv, bv_, wo, bo_):
    shared = {
        "wqt": np.ascontiguousarray(wq.T.astype(np.float32)),
        "wkt": np.ascontiguousarray(wk.T.astype(np.float32)),
        "wvt": np.ascontiguousarray(wv.T.astype(np.float32)),
        "wot": np.ascontiguousarray(
            wo.transpose(2, 3, 1, 0).reshape(9, C, C).astype(np.float32)),
        "bq": np.ascontiguousarray(bq_.astype(np.float32)),
        "bk": np.ascontiguousarray(bk_.astype(np.float32)),
        "bv": np.ascontiguousarray(bv_.astype(np.float32)),
        "bo": np.ascontiguousarray(bo_.astype(np.float32)),
    }
    x3 = np.ascontiguousarray(x.reshape(2 * T, C, PIX).astype(np.float32))
    in_maps = []
    for core in range(NCORES):
        v, f = divmod(core, T)
        perm = [f] + [g for g in range(T) if g != f]
        m = dict(shared)
        m["xv"] = np.ascontiguousarray(x3[v * T:(v + 1) * T][perm])
        in_maps.append(m)
    return in_maps


def kernel(**inputs):
    from concourse.bass_utils import run_bass_kernel_spmd

    x = np.asarray(inputs["x"], dtype=np.float32)
    in_maps = make_in_maps(
        x, np.asarray(inputs["wq"]), np.asarray(inputs["bq"]),
        np.asarray(inputs["wk"]), np.asarray(inputs["bk"]),
        np.asarray(inputs["wv"]), np.asarray(inputs["bv"]),
        np.asarray(inputs["wo"]), np.asarray(inputs["bo"]))
    nc = _get_nc()
    res = run_bass_kernel_spmd(nc, in_maps, core_ids=list(range(NCORES)))
    outs = [res.results[c]["out"].reshape(C, H, W) for c in range(NCORES)]
    return np.stack(outs).astype(np.float32)
